# revision 7
# baseline (speedup 1.0000x reference)
"""Trainium2 Bass kernel for batched linear attention (no softmax), v2.

Reference computation (per batch b):
    q = x Wq^T + bq ; k = x Wk^T + bk ; v = x Wv^T + bv
    out = (q k^T / sqrt(D)) v

With augmented x' = [x | 1 | 0] [S, 770] and Aw = [W^T ; b ; 0] [770, 768]:
    out = x' Aq (Ak^T G' Av) / sqrt(D)      G' = x'^T x'
        = x' W1 G' Av                       W1 = Aq Ak^T / sqrt(D)  (host)
so the device computes, per core:
    G'  = x'^T x'          (symmetric: upper trapezoid on PE + mirrors)
    GV  = G' Avh           [770, 384]   (Avh = Av[:, e-half])
    P2  = W1 GV            [770, 384]
    out = x' P2            [S, 384]     (bias via P2 row 768 broadcast)

Sharding: 8 cores = 4 batches x 2 output-column halves. Stage-1 G' is
pair-redundant (the S-coupling makes that the cheapest no-communication
split); everything downstream is half-width. Host precomputing W1 removes
one full GEMM versus the Aq (Ak^T G' Av) order.

All PE operands are bf16 (1 cycle/row at any width, half DMA bytes);
PSUM accumulation is fp32 and the final output is fp32.

Scheduling notes (from HW bisection, not the cost model):
- input DMAs are grouped (multiple 128-row tiles per transfer) on the
  qSP HWDGE queue: per-DMA queue-generation overhead (~0.6us) adds up;
- output rows are staged into [128, 8*384] stripes and leave on the
  Activation HWDGE queue so they never contend with input transfers;
- PSUM->SBUF evictions alternate DVE/Activation; all PE transposes go
  to their own banks back-to-back (an eviction ping-pong here showed up
  as ~10us of PE stall);
- GV/P2 run their 6 dense K-passes first and the 2-row augmented pass
  last, so the g_row assembly is off the critical path; the P2 tail
  rows + bias broadcast are computed before the P2 body so stage-4's
  adds never wait on them.
"""

import math

import numpy as np

B, S, D = 4, 4096, 768
DA = D + 2          # augmented dim: ones col at 768, zeros col at 769
P = 128
EH = D // 2         # per-core output-column half (384)
N_CORES = 8
NT_S = S // P       # 32 sequence tiles
ND = D // P         # 6 blocks of 128 over D

CONFIG = {"reps": 1, "upto": 9}

_CACHE = {}

# stage-1 symmetric trapezoid jobs: (md, col0, width). md row-block covers
# cols [md*128, 770); chunks <= 512 (PSUM bank f32). PSUM accumulation
# groups are bank-granular, so each job owns a bank: 8 jobs in pass 1,
# md5's 130-wide job runs as pass 2 once a bank frees (same total cycles;
# all x tiles stay resident in SBUF).
SYM_PASSES = [
    [(0, 0, 512), (0, 512, 258), (1, 128, 512), (1, 640, 130),
     (2, 256, 512), (2, 768, 2), (3, 384, 386), (4, 512, 258)],
    [(5, 640, 130)],
]

# mirror blocks (md, nb): md > nb, in GV-consumption order for mb-descending
# chains (GV block (mb, kt) with kt > mb reads mirror (kt, mb)).
MIRROR_ORDER = [(5, 4), (4, 3), (5, 3), (3, 2), (4, 2), (5, 2),
                (2, 1), (3, 1), (4, 1), (5, 1),
                (1, 0), (2, 0), (3, 0), (4, 0), (5, 0)]


def _build_nc(reps=1, upto=9):
    import concourse.bacc as bacc
    import concourse.mybir as mybir
    import concourse.tile as tile
    from concourse.masks import make_identity
    from contextlib import ExitStack

    f32 = mybir.dt.float32
    bf16 = mybir.dt.bfloat16

    nc = bacc.Bacc("TRN2", target_bir_lowering=False, debug=False,
                   num_devices=N_CORES)

    xs_t = nc.dram_tensor("xs", [S, DA], bf16, kind="ExternalInput")
    xt_t = nc.dram_tensor("xt", [D, S], bf16, kind="ExternalInput")
    avh_t = nc.dram_tensor("avh", [DA, EH], bf16, kind="ExternalInput")
    w1t_t = nc.dram_tensor("w1t", [DA, DA], bf16, kind="ExternalInput")
    out_t = nc.dram_tensor("out", [EH, S], bf16, kind="ExternalOutput")
    xs, xt, avh, w1t, outd = (t.ap() for t in
                              (xs_t, xt_t, avh_t, w1t_t, out_t))

    def mm(ps, lh, rh, start, stop):
        nc.tensor.matmul(ps, lhsT=lh, rhs=rh, start=start, stop=stop)

    def group_dma(engine, dst_tile, dst_col0, src, row0, nrows, width):
        """One DMA moving nrows/128 row-tiles of `width` cols into
        consecutive dst column blocks."""
        j = nrows // P
        dst = dst_tile[:, dst_col0:dst_col0 + j * width].rearrange(
            "p (j c) -> p j c", j=j)
        s = src[row0:row0 + nrows, :].rearrange("(j p) c -> p j c", p=P)
        engine.dma_start(out=dst, in_=s)

    def body(tc):
        es = ExitStack()
        stage = [0]

        def done():
            stage[0] += 1
            return stage[0] > upto

        evict_flip = [0]

        def evict(dst, src):
            if evict_flip[0] % 2 == 0:
                nc.vector.tensor_copy(dst, src)
            else:
                nc.scalar.copy(dst, src)
            evict_flip[0] += 1

        pp = es.enter_context(tc.tile_pool(name="persist", bufs=1))
        ident = pp.tile([P, P], bf16, name="ident", tag="ident")
        ones1 = pp.tile([1, 2], bf16, name="ones1", tag="ones1")
        ones1f = pp.tile([1, 2], f32, name="ones1f", tag="ones1f")
        idf = pp.tile([P, P], f32, name="idf", tag="idf")
        zrow = pp.tile([2, DA], f32, name="zrow", tag="zrow")
        corner = pp.tile([1, 2], f32, name="corner", tag="corner")
        # G' rows d'<768 as 6 partition-tiles: g_sb[p, t*DA+j] = G'[t*128+p, j]
        g_sb = pp.tile([P, ND * DA], bf16, name="g_sb", tag="g_sb")
        g_row = pp.tile([2, DA], bf16, name="g_row", tag="g_row")
        make_identity(nc, idf)
        nc.any.memset(zrow[0:2, :], 0.0)
        nc.any.memset(corner[0:1, 0:1], float(S))
        nc.any.memset(corner[0:1, 1:2], 0.0)
        nc.any.memset(ones1f[0:1, :], 1.0)
        nc.vector.tensor_copy(ones1[0:1, :], ones1f[0:1, :])
        nc.vector.tensor_copy(ident[:, :], idf[:, :])

        # grouped input DMAs, all on the qSP HWDGE queue, in consumption
        # order: xs (stage 1), avh (GV), w1t (P2), xt (stage 4).
        xp = es.enter_context(tc.tile_pool(name="xp", bufs=1))
        wp = es.enter_context(tc.tile_pool(name="wp", bufs=1))
        xtp = es.enter_context(tc.tile_pool(name="xtp", bufs=1))
        # xs group sizes: stage 1 is PE-paced once running, so its finish
        # time tracks its start time -- a small first group (2 tiles,
        # ~1.8us) lets the PE start ~5us earlier than one 8-tile group.
        XS_GROUPS = [2, 6, 8, 8, 8]
        xg_of, xoff_of = [], []
        row0 = 0
        for gi, gsz in enumerate(XS_GROUPS):
            t = xp.tile([P, gsz * DA], bf16, name=f"xg{gi}", tag=f"xg{gi}")
            group_dma(nc.sync, t, 0, xs, row0 * P, gsz * P, DA)
            for j in range(gsz):
                xg_of.append(t)
                xoff_of.append(j)
            row0 += gsz

        def x_tile(i):
            return xg_of[i][:, xoff_of[i] * DA:(xoff_of[i] + 1) * DA]

        avh_sb = wp.tile([P, ND * EH], bf16, name="avh_sb", tag="avh_sb")
        avh_row = wp.tile([2, EH], bf16, name="avh_row", tag="avh_row")
        group_dma(nc.sync, avh_sb, 0, avh, 0, ND * P, EH)
        nc.sync.dma_start(out=avh_row[0:2, :], in_=avh[768:770, :])
        w1t_sb = wp.tile([P, ND * DA], bf16, name="w1t_sb", tag="w1t_sb")
        w1t_row = wp.tile([2, DA], bf16, name="w1t_row", tag="w1t_row")
        group_dma(nc.sync, w1t_sb, 0, w1t, 0, 3 * P, DA)
        group_dma(nc.sync, w1t_sb, 3 * DA, w1t, 3 * P, 3 * P, DA)
        nc.sync.dma_start(out=w1t_row[0:2, :], in_=w1t[768:770, :])
        xt_sb = xtp.tile([P, ND * S], bf16, name="xt_sb", tag="xt_sb")
        group_dma(nc.sync, xt_sb, 0, xt, 0, 3 * P, S)
        group_dma(nc.sync, xt_sb, 3 * S, xt, 3 * P, 3 * P, S)

        if done():
            es.close()
            return
        # ---- Stage 1: G' = x'^T x' (upper trapezoid, 8+1 PSUM jobs) ----
        with tc.tile_pool(name="gps", bufs=8, space="PSUM") as gpsp:
            for pass_jobs in SYM_PASSES:
                pss = {(md, c0): gpsp.tile([P, 512], f32,
                                           name=f"gps_{md}_{c0}", tag="gps")
                       for (md, c0, cw) in pass_jobs}
                for st in range(NT_S):
                    for (md, c0, cw) in pass_jobs:
                        mm(pss[(md, c0)][:, :cw],
                           x_tile(st)[:, md * P:(md + 1) * P],
                           x_tile(st)[:, c0:c0 + cw],
                           start=(st == 0), stop=(st == NT_S - 1))
                for (md, c0, cw) in pass_jobs:
                    evict(g_sb[:, md * DA + c0: md * DA + c0 + cw],
                          pss[(md, c0)][:, :cw])

        if done():
            es.close()
            return
        # mirrors + g_row as a deferred emission (called from the GV
        # phase after mb5's dense run): all transposes back-to-back on PE.
        def emit_transposes():
            tps_cm = tc.tile_pool(name="tps", bufs=4, space="PSUM")
            tpsp = tps_cm.__enter__()
            nc.scalar.copy(g_row[0:2, :], zrow[0:2, :])
            for t in range(ND):
                pr = tpsp.tile([P, 512], bf16, name=f"tp{t}", tag="tps")
                nc.tensor.matmul(
                    pr[0:1, 0:P],
                    lhsT=g_sb[:, t * DA + 768: t * DA + 769],
                    rhs=ident[:, :], is_transpose=True,
                    start=True, stop=True)
                evict(g_row[0:1, t * P:(t + 1) * P], pr[0:1, 0:P])
            nc.scalar.copy(g_row[0:1, 768:770], corner[0:1, 0:2])
            for (md, nb) in MIRROR_ORDER:
                pt = tpsp.tile([P, 512], bf16, name=f"tm{md}_{nb}",
                               tag="tps")
                nc.tensor.matmul(
                    pt[:, 0:P],
                    lhsT=g_sb[:, nb * DA + md * P: nb * DA + (md + 1) * P],
                    rhs=ident[:, :], is_transpose=True,
                    start=True, stop=True)
                evict(g_sb[:, md * DA + nb * P: md * DA + (nb + 1) * P],
                      pt[:, 0:P])
            tps_cm.__exit__(None, None, None)

        if done():
            es.close()
            return
        mats = es.enter_context(tc.tile_pool(name="mats", bufs=1))
        gv_sb = mats.tile([P, ND * EH], bf16, name="gv_sb", tag="gv_sb")
        gv_row = mats.tile([2, EH], bf16, name="gv_row", tag="gv_row")
        p2_sb = mats.tile([P, ND * EH], bf16, name="p2_sb", tag="p2_sb")
        p2_row = mats.tile([2, EH], bf16, name="p2_row", tag="p2_row")
        biast = mats.tile([P, 3], f32, name="biast", tag="biast")

        # ---- Stage 2: GV = G' Avh ([770, 384]) ----
        # per-mb complete chains, staggered so every bank evicts as soon
        # as its augmented pass retires; mb5's dense run is emitted before
        # the transposes (it needs no mirrors) to cover their latency.
        with tc.tile_pool(name="ps2", bufs=4, space="PSUM") as ps2:
            pss = {}

            def gv_dense(mb):
                pss[mb] = ps2.tile([P, EH], f32, name=f"gvps{mb}",
                                   tag="gvps")
                for kt in range(ND):
                    mm(pss[mb][:, :],
                       g_sb[:, kt * DA + mb * P: kt * DA + (mb + 1) * P],
                       avh_sb[:, kt * EH:(kt + 1) * EH],
                       start=(kt == 0), stop=False)

            def gv_aug(mb):
                mm(pss[mb][:, :], g_row[0:2, mb * P:(mb + 1) * P],
                   avh_row[0:2, :], start=False, stop=True)
                evict(gv_sb[:, mb * EH:(mb + 1) * EH], pss[mb][:, :])

            gv_dense(5)
            emit_transposes()
            gv_aug(5)
            for mb in [4, 3, 2, 1, 0]:
                gv_dense(mb)
                gv_aug(mb)
            psr = ps2.tile([2, EH], f32, name="gvpsr", tag="gvps")
            for kt in range(ND):
                mm(psr[0:2, :],
                   g_sb[:, kt * DA + 768: kt * DA + 770],
                   avh_sb[:, kt * EH:(kt + 1) * EH],
                   start=(kt == 0), stop=False)
            mm(psr[0:2, :], g_row[0:2, 768:770], avh_row[0:2, :],
               start=False, stop=True)
            evict(gv_row[0:2, :], psr[0:2, :])

        if done():
            es.close()
            return
        # ---- Stage 3: P2 = W1 GV ([770, 384]) ----
        # kt-passes descending (matches GV eviction order); per-mb chains
        # staggered with immediate eviction; aug pass last per chain; the
        # three tiny bias-column chains share one recycled bank at the end.
        with tc.tile_pool(name="ps3", bufs=4, space="PSUM") as ps3, \
             tc.tile_pool(name="psb", bufs=2, space="PSUM") as psb:
            psr = psb.tile([2, EH], f32, name="p2psr", tag="bps")
            for i, kt in enumerate([5, 4, 3, 2, 1, 0]):
                mm(psr[0:2, :], w1t_sb[:, kt * DA + 768: kt * DA + 770],
                   gv_sb[:, kt * EH:(kt + 1) * EH],
                   start=(i == 0), stop=False)
            mm(psr[0:2, :], w1t_row[0:2, 768:770], gv_row[0:2, :],
               start=False, stop=True)
            nc.scalar.copy(p2_row[0:2, :], psr[0:2, :])
            # cross-partition move: out[m, 0] = sum_k lhsT[k, m] rhs[k, 0]
            # with K=1 and rhs = [[1.0]] turns the bias row into a column.
            for eb in range(3):
                bps = psb.tile([P, 512], f32, name=f"bps{eb}", tag="bps")
                mm(bps[0:P, 0:1], p2_row[0:1, eb * P:(eb + 1) * P],
                   ones1[0:1, 0:1], start=True, stop=True)
                nc.vector.tensor_copy(biast[:, eb:eb + 1], bps[0:P, 0:1])
            mbs = list(range(ND))
            pss = {}
            for mb in mbs:
                pss[mb] = ps3.tile([P, EH], f32, name=f"p2ps{mb}",
                                   tag="p2ps")
                for i, kt in enumerate([5, 4, 3, 2, 1, 0]):
                    mm(pss[mb][:, :],
                       w1t_sb[:, kt * DA + mb * P: kt * DA + (mb + 1) * P],
                       gv_sb[:, kt * EH:(kt + 1) * EH],
                       start=(i == 0), stop=False)
                mm(pss[mb][:, :], w1t_row[0:2, mb * P:(mb + 1) * P],
                   gv_row[0:2, :], start=False, stop=True)
                evict(p2_sb[:, mb * EH:(mb + 1) * EH], pss[mb][:, :])

        if done():
            es.close()
            return
        # ---- Stage 4: out^T = P2^T x'^T (+ bias via per-partition add) ----
        # 4-chunk segments, kt-outer inside a segment: the stationary lhsT
        # (a P2 block) is reused across 4 consecutive matmuls; evictions
        # alternate Activation (fused bias add) and DVE (tensor_scalar_add)
        # so PSUM banks recycle at double rate; each half-stripe leaves as
        # its own DMA so the output drain starts earlier.
        NSC = S // 512
        with tc.tile_pool(name="osb", bufs=2) as osbp, \
             tc.tile_pool(name="ps5", bufs=8, space="PSUM") as ps5:
            for eb in range(3):
                stripe = osbp.tile([P, S], bf16, name=f"st{eb}", tag="osb")
                for half in range(2):
                    scs = range(half * 4, half * 4 + 4)
                    pss = {sc: ps5.tile([P, 512], f32, name=f"ops{eb}_{sc}",
                                        tag="ops") for sc in scs}
                    for i, kt in enumerate(range(ND)):
                        lh = p2_sb[:, kt * EH + eb * P:
                                   kt * EH + (eb + 1) * P]
                        for sc in scs:
                            mm(pss[sc][:, :], lh,
                               xt_sb[:, kt * S + sc * 512:
                                     kt * S + (sc + 1) * 512],
                               start=(i == 0), stop=(i == ND - 1))
                    for j, sc in enumerate(scs):
                        if j % 2 == 0:
                            nc.scalar.add(stripe[:, sc * 512:(sc + 1) * 512],
                                          pss[sc][:, :], biast[:, eb:eb + 1])
                        else:
                            nc.vector.tensor_scalar_add(
                                stripe[:, sc * 512:(sc + 1) * 512],
                                pss[sc][:, :], biast[:, eb:eb + 1])
                    nc.sync.dma_start(
                        out=outd[eb * P:(eb + 1) * P,
                                 half * 2048:(half + 1) * 2048],
                        in_=stripe[:, half * 2048:(half + 1) * 2048])
        es.close()

    with tile.TileContext(nc) as tc:
        if reps > 1:
            with tc.For_i(0, reps):
                body(tc)
        else:
            body(tc)

    nc.compile()
    return nc


def get_nc():
    key = ("nc", CONFIG["reps"], CONFIG["upto"])
    if key not in _CACHE:
        _CACHE[key] = _build_nc(reps=CONFIG["reps"], upto=CONFIG["upto"])
    return _CACHE[key]


def make_in_maps(x, Wq, bq, Wk, bk, Wv, bv):
    import ml_dtypes
    bf16 = ml_dtypes.bfloat16
    f32 = np.float32
    x = np.asarray(x, dtype=f32)
    scale = 1.0 / math.sqrt(D)
    z1 = np.zeros((1, D), f32)
    aq = np.concatenate(
        [np.asarray(Wq, f32).T, np.asarray(bq, f32)[None, :], z1], 0)
    ak = np.concatenate(
        [np.asarray(Wk, f32).T, np.asarray(bk, f32)[None, :], z1], 0)
    av = np.concatenate(
        [np.asarray(Wv, f32).T, np.asarray(bv, f32)[None, :], z1], 0)
    w1 = (aq.astype(np.float64) @ ak.astype(np.float64).T) * scale
    w1t = np.ascontiguousarray(w1.T).astype(bf16)
    in_maps = []
    for core in range(N_CORES):
        b, h = core // 2, core % 2
        xa = np.concatenate(
            [x[b], np.ones((S, 1), f32), np.zeros((S, 1), f32)], 1)
        xsv = np.ascontiguousarray(xa).astype(bf16)
        xtv = np.ascontiguousarray(x[b].T).astype(bf16)
        avh = np.ascontiguousarray(av[:, h * EH:(h + 1) * EH]).astype(bf16)
        in_maps.append({"xs": xsv, "xt": xtv, "avh": avh, "w1t": w1t})
    return in_maps


def gather_out(results):
    out = np.empty((B, S, D), np.float32)
    for core in range(N_CORES):
        b, h = core // 2, core % 2
        out[b, :, h * EH:(h + 1) * EH] = \
            results[core]["out"].astype(np.float32).T
    return out


def run(in_maps, trace=False, **kwargs):
    from concourse import bass_utils
    nc = get_nc()
    return bass_utils.run_bass_kernel_spmd(nc, in_maps, list(range(N_CORES)),
                                           trace=trace, **kwargs)


def kernel(x, Wq, bq, Wk, bk, Wv, bv):
    in_maps = make_in_maps(x, Wq, bq, Wk, bk, Wv, bv)
    res = run(in_maps)
    return gather_out(res.results)
